# revision 41
# baseline (speedup 1.0000x reference)
"""Trainium2 Bass kernel for the CdfgReader GNN message-passing problem.

Reference computation (shapes hardcoded):
    G, N, F, H, B, L = 4, 1024, 256, 256, 32, 4
    X = batch_xs[graph_idx]          # [B, N, F]
    A = batch_as[graph_idx]          # [B, N, N]
    x = relu(X @ w_in + b_in)
    res = x
    for i in range(L-1): x = relu(A @ x @ gcn_w[i] + gcn_b[i])
    x = tanh(A @ x @ gcn_w[L-1] + gcn_b[L-1])
    x = x + res
    out[b] = masked_mean_over_nodes(x[b], cp_mask[b])   # [B, H]

Key structural insight: the whole forward up to the final masked mean depends
only on which of the G=4 distinct graphs an example selects — so we compute
the forward once per distinct graph (4 graphs) instead of once per example
(32 examples), an 8x FLOP reduction. The per-example masked mean then becomes
a tiny [B,N]x[N,H] matmul against a host-built selection matrix.

Sharding: graph-parallel — core g (g in 0..3) computes graph g's full forward
plus its [B,H] partial of the output; cores 4..7 run the same program on
zeros. The host sums the (disjoint) partials and divides by the mask counts.

Per-core device program (measured max-rel-err ~1.6e-3 end to end — the
output is dominated by the bf16 residual path, so the whole GCN stack runs
in fp8e4m3 with per-tensor pre-scales and DoubleRow matmuls, 2 MACs/cell):
    x0  = relu(XT.T @ w_in)            bf16, 16 matmuls   (lhsT = XT)
    per layer (fp8 DoubleRow):
        zT = (x.T @ AT)                16 matmuls, contraction 256/matmul
        x' = act(zT.T @ W_l)           8 matmuls
    out_partial = R.T@x4 + R.T@x0      bf16, 16 matmuls into one PSUM
The alternating lhsT choice (x -> zT -> x) makes the chain transpose-free;
all activation/copy stages alternate between ACT and DVE so neither gates
the PE stream.
"""

import numpy as np
import ml_dtypes

G, N, F, H, B, L = 4, 1024, 256, 256, 32, 4
N_CORES = 8
NT = N // 128          # 8 node tiles
FT = F // 128          # 2 feature tiles
HT = H // 128          # 2 hidden tiles
NCHUNK = 512           # stage-A moving free dim (one fp32 PSUM bank)
XS = [8.0, 64.0, 256.0, 1024.0]   # fp8 pre-scales for x entering stage A
ASCALE = 1024.0                   # fp8 pre-scale for A^T (entries ~U[0,1]/N)
ZS = [1.0 / 64, 1.0 / 128, 1.0 / 128, 1.0 / 128]  # fp8 pre-scales for zT
GS = 8.0                          # fp8 pre-scale for gcn_w

_CACHE = {}


def _split_multi_waits(nc):
    """The walrus build in this container accepts at most ONE sync wait per
    instruction, while Tile's sem-assignment emits up to ~3. Engines execute
    their instruction stream in order, so an instruction's extra waits can be
    hoisted onto same-engine NoOps inserted immediately before it."""
    import concourse.mybir as mybir

    n = 0
    for f in nc.m.functions:
        for bb in f.blocks:
            out = []
            changed = False
            for ins in bb.instructions:
                si = ins.sync_info
                if si is not None and si.on_wait and len(si.on_wait) > 1:
                    waits = list(si.on_wait)
                    for w in waits[:-1]:
                        nop = mybir.InstNoOp(
                            name=f"wsplit_{n}", engine=ins.engine)
                        n += 1
                        nop.sync_info = mybir.SyncInfo(on_wait=[w], on_update=[])
                        out.append(nop)
                    si.on_wait = [waits[-1]]
                    changed = True
                out.append(ins)
            if changed:
                bb.instructions = out
    return nc


def _build_nc(use_bias):
    import concourse.bass as bass
    import concourse.mybir as mybir

    dt = mybir.dt.bfloat16
    d8 = mybir.dt.float8e4
    f32 = mybir.dt.float32
    AF = mybir.ActivationFunctionType

    nc = bass.Bass(enable_partition_id=False, num_swdge_queues=4)
    # DRAM I/O (per core). All inputs are pre-tiled on the host into
    # [128, ...] partition-major contiguous layouts so each DMA moves
    # maximal contiguous runs (strided descriptors measured ~2-4x slower).
    xt_d = nc.dram_tensor("xt", [2, 128, FT * N // 2], dt, kind="ExternalInput")
    at_d = nc.dram_tensor("at", [4, 128, 2 * N], d8, kind="ExternalInput")
    w_in_d = nc.dram_tensor("w_in", [128, FT * H], dt, kind="ExternalInput")
    gw_d = nc.dram_tensor("gw", [128, L * HT * H], d8, kind="ExternalInput")
    r_d = nc.dram_tensor("r", [128, NT * B], dt, kind="ExternalInput")
    if use_bias:
        # biases pre-broadcast over partitions on host: [L+1, 128, H]
        bias_d = nc.dram_tensor("bias", [L + 1, 128, H], f32, kind="ExternalInput")
    out_d = nc.dram_tensor("out", [B, H], f32, kind="ExternalOutput")

    from concourse.tile import TileContext

    with TileContext(nc) as tc:
        import contextlib

        with contextlib.ExitStack() as ctx:
            consts = ctx.enter_context(tc.tile_pool(name="consts", bufs=1))
            xpool = ctx.enter_context(tc.tile_pool(name="x", bufs=1))
            zpool = ctx.enter_context(tc.tile_pool(name="z", bufs=2))
            opool = ctx.enter_context(tc.tile_pool(name="o", bufs=2))
            psA = ctx.enter_context(tc.tile_pool(name="psA", bufs=4, space="PSUM"))
            psB = ctx.enter_context(tc.tile_pool(name="psB", bufs=4, space="PSUM"))

            # ---- loads: few big DMAs (each dma_start costs ~0.6us of issue
            # time on its engine, so 28 small ones would serialize ~18us).
            # Spread across idle engines so issues run in parallel; keep PE
            # free so matmuls start the moment xt+wi land. ----
            # xt split in two column-halves (separate dep units): the input
            # layer's first 4 m-tiles only need columns 0:512
            # xt halves on two independent DMA rings (sync HWDGE +
            # gpsimd 4-queue SWDGE) — a single ring caps ~100 GB/s
            xt_h = []
            for hn, eng in ((0, nc.sync), (1, nc.gpsimd)):
                t = consts.tile([128, FT, N // 2], dt, tag=f"xt{hn}", name=f"xt_h{hn}")
                eng.dma_start(out=t.rearrange("p t n -> p (t n)"), in_=xt_d[hn])
                xt_h.append(t)

            wi_b = consts.tile([128, FT, H], dt, tag="wi", name="wi_b")
            nc.scalar.dma_start(out=wi_b.rearrange("p t h -> p (t h)"),
                                in_=w_in_d[:, :])
            w_in = [wi_b[:, k, :] for k in range(FT)]

            # A^T in 4 independent tiles (separate dep-tracking units) so
            # layer-0 stage A starts as soon as the first group lands
            at_g = []
            at_engines = [nc.gpsimd, nc.scalar, nc.sync, nc.gpsimd]
            for g in range(4):
                t = consts.tile([128, 2, N], d8, tag=f"at{g}", name=f"at_g{g}")
                at_engines[g].dma_start(out=t.rearrange("p t n -> p (t n)"),
                                        in_=at_d[g])
                at_g.append(t)

            gw_b = consts.tile([128, L * HT, H], d8, tag="gw", name="gw_b")
            nc.scalar.dma_start(out=gw_b.rearrange("p t h -> p (t h)"),
                                in_=gw_d[:, :])

            r_b = consts.tile([128, NT, B], dt, tag="r", name="r_b")
            nc.sync.dma_start(out=r_b.rearrange("p t b -> p (t b)"), in_=r_d[:, :])
            r = [r_b[:, k, :] for k in range(NT)]
            if use_bias:
                bias = [consts.tile([128, H], f32, tag=f"b{i}", name=f"b{i}") for i in range(L + 1)]
                for i in range(L + 1):
                    nc.sync.dma_start(out=bias[i], in_=bias_d[i])

            # ---- input dense layer: x0 = relu(X @ w_in + b_in) ----
            # bf16 copy feeds the residual; fp8 copy (scaled by XS[0]) feeds
            # layer-0 stage A. fp8 stage A is accuracy-free here because the
            # output is dominated by the residual path (verified 1.2e-3).
            # psums live in [128, 2, H] pairs (one bank) so one wide
            # activation drains two m-tiles at once
            x0 = [xpool.tile([128, 2, H], dt, tag=f"x0_{g}", name=f"x0_{g}")
                  for g in range(NT // 2)]
            x08 = [xpool.tile([128, 2, H], d8, tag=f"x8in_{g}", name=f"x08_{g}")
                   for g in range(NT // 2)]
            for g in range(NT // 2):
                ps = psB.tile([128, 2, H], f32, tag="psB", name="psB_t")
                for j in range(2):
                    m = 2 * g + j
                    for k in range(FT):
                        xm = xt_h[m // 4][:, k, 128 * (m % 4):128 * (m % 4 + 1)]
                        nc.tensor.matmul(ps[:, j, :], xm, w_in[k],
                                         start=(k == 0), stop=(k == FT - 1))
                if use_bias:
                    nc.vector.tensor_add(ps[:, 0, :], ps[:, 0, :], bias[0])
                    nc.vector.tensor_add(ps[:, 1, :], ps[:, 1, :], bias[0])
                nc.scalar.activation(out=x0[g], in_=ps, func=AF.Relu)
                nc.vector.tensor_scalar(
                    out=x08[g], in0=ps, scalar1=XS[0], scalar2=0.0,
                    op0=mybir.AluOpType.mult, op1=mybir.AluOpType.max)

            # ---- GCN layers ----
            NC = N // NCHUNK      # dst chunks per row (2)
            MPC = NCHUNK // 128   # node tiles per chunk (4)
            x_cur = x08
            for layer in range(L):
                # stage A: zT[h, dst] = sum_src x[src, h] * AT[src, dst].
                # c (dst chunk) is the outer loop and each (h, c) gets its own
                # SBUF tile, so stage B's m-tiles in chunk c unblock while
                # stage A still streams chunk c+1 on the PE — no PE bubble
                # waiting on the PSUM->SBUF copies.
                # DoubleRow fp8: out[m,n] = sum_k sum_j lhsT[k,j,m]*rhs[k,j,n]
                # contracts 256 src nodes per matmul (2 fp8 weights per cell)
                zT = [zpool.tile([128, HT, NCHUNK], d8, tag=f"zT_{c}",
                                 name=f"zT_{layer}_{c}") for c in range(NC)]
                # g-major emission: all 4 accumulation chains advance together
                # across the 4 PSUM banks, so the in-order PE stream never
                # head-of-line blocks on the last x8 pair (or, in layer 0, on
                # the last A^T DMA group) while other chains' work is ready
                chains = [(c, h) for c in range(NC) for h in range(HT)]
                pss = {ch: psA.tile([128, NCHUNK], f32, tag="psA", name="psA_t")
                       for ch in chains}
                for g in range(NT // 2):
                    for c, h in chains:
                        nc.tensor.matmul(
                            pss[(c, h)],
                            x_cur[g][:, :, 128 * h:128 * (h + 1)],
                            at_g[g][:, :, NCHUNK * c:NCHUNK * (c + 1)],
                            start=(g == 0), stop=(g == NT // 2 - 1),
                            perf_mode=mybir.MatmulPerfMode.DoubleRow)
                for c, h in chains:
                    # scaled fp8 copy; alternate engines so copies land in
                    # parallel instead of queueing on one engine
                    if h == 0:
                        nc.scalar.activation(out=zT[c][:, h, :], in_=pss[(c, h)],
                                             func=AF.Copy, scale=ZS[layer])
                    else:
                        nc.vector.tensor_scalar_mul(
                            out=zT[c][:, h, :], in0=pss[(c, h)], scalar1=ZS[layer])
                # stage B: x'[dst, h'] = act(sum_h zT[h, dst] * W[h, h'] + b).
                # gw was pre-divided by 1024*XS[layer] on the host, undoing the
                # fp8 pre-scales. Layers 0..2 write fp8 scaled by XS[layer+1]
                # (next stage A operand); the last layer writes bf16 for the
                # residual add.
                last = layer == L - 1
                if last:
                    x_nxt = [xpool.tile([128, 2, H], dt, tag=f"xl_{g}",
                                        name=f"xl_{g}") for g in range(NT // 2)]
                else:
                    x_nxt = [xpool.tile([128, 2, H], d8,
                                        tag=f"x8_{layer % 2}_{g}",
                                        name=f"x8_{layer}_{g}")
                             for g in range(NT // 2)]
                # psum here = (ASCALE*XS[l]*ZS[l]*GS) * (z @ W); corr undoes it
                corr = 1.0 / (ASCALE * XS[layer] * ZS[layer] * GS)
                for g in range(NT // 2):
                    ps = psB.tile([128, 2, H], f32, tag="psB", name="psB_t")
                    for j in range(2):
                        m = 2 * g + j
                        c, mc = m // MPC, m % MPC
                        nc.tensor.matmul(
                            ps[:, j, :], zT[c][:, :, 128 * mc:128 * (mc + 1)],
                            gw_b[:, 2 * layer:2 * layer + 2, :],
                            start=True, stop=True,
                            perf_mode=mybir.MatmulPerfMode.DoubleRow)
                    if use_bias:
                        nc.vector.tensor_add(ps[:, 0, :], ps[:, 0, :], bias[layer + 1])
                        nc.vector.tensor_add(ps[:, 1, :], ps[:, 1, :], bias[layer + 1])
                    if last:
                        nc.scalar.activation(out=x_nxt[g], in_=ps, func=AF.Tanh,
                                             scale=corr)
                    elif g % 2 == 0:
                        nc.scalar.activation(out=x_nxt[g], in_=ps,
                                             func=AF.Relu,
                                             scale=XS[layer + 1] * corr)
                    else:
                        nc.vector.tensor_scalar(
                            out=x_nxt[g], in0=ps,
                            scalar1=XS[layer + 1] * corr, scalar2=0.0,
                            op0=mybir.AluOpType.mult, op1=mybir.AluOpType.max)
                x_cur = x_nxt

            # ---- masked-sum matmul; residual folded in:
            # out = R.T@(x4 + x0) = R.T@x4 + R.T@x0 accumulated in one PSUM.
            # Interleave the x0 terms (ready long ago) between the x4 terms
            # (each gated by its tanh) so the PE never idles on ACT. ----
            ps = psB.tile([32, H], f32, tag="psB", name="psB_out")
            for k in range(NT):
                nc.tensor.matmul(ps, r[k][:, :], x0[k // 2][:, k % 2, :],
                                 start=(k == 0), stop=False)
                nc.tensor.matmul(ps, r[k][:, :], x_cur[k // 2][:, k % 2, :],
                                 start=False, stop=(k == NT - 1))
            out_sb = opool.tile([32, H], f32, tag="out", name="out_sb")
            nc.scalar.copy(out=out_sb, in_=ps)
            nc.sync.dma_start(out=out_d[:, :], in_=out_sb)

    _split_multi_waits(nc)
    return nc


def _get_nc(use_bias):
    key = ("nc", use_bias)
    if key not in _CACHE:
        _CACHE[key] = _build_nc(use_bias)
    return _CACHE[key]


def _prepare_in_maps(batch_xs, batch_as, w_in, b_in, gcn_w, gcn_b,
                     graph_idx, cp_mask, use_bias):
    bf16 = ml_dtypes.bfloat16
    fp8 = ml_dtypes.float8_e4m3fn
    mask_f = cp_mask.astype(np.float32)                     # [B, N]

    def ptile(a, inner=128):
        # [T*128, W] -> [128, T*W] partition-major contiguous
        tw = a.reshape(-1, inner, a.shape[-1])
        return np.ascontiguousarray(
            tw.transpose(1, 0, 2).reshape(inner, -1))

    w_in_b = ptile(w_in.astype(bf16))                       # [128, 2*H]
    gw_b = ptile((gcn_w * GS).astype(fp8).reshape(L * H, H))  # [128, 8*H]
    if use_bias:
        # gcn biases add into the scaled stage-B PSUM domain
        bscale = np.array([1.0] + [ASCALE * XS[i] * ZS[i] * GS for i in range(L)],
                          np.float32)
        bias_full = np.concatenate(
            [b_in[None, :], gcn_b], axis=0).astype(np.float32) * bscale[:, None]
        bias_bcast = np.ascontiguousarray(
            np.broadcast_to(bias_full[:, None, :], (L + 1, 128, H)).copy())

    in_maps = []
    for c in range(N_CORES):
        if c < G:
            g = c
            xtf = batch_xs[g].T.astype(bf16)
            xt = np.stack([ptile(np.ascontiguousarray(xtf[:, :N // 2])),
                           ptile(np.ascontiguousarray(xtf[:, N // 2:]))])
            atf = (batch_as[g].T * ASCALE).astype(fp8)             # [N, N]
            at = np.stack([ptile(atf[256 * gg:256 * (gg + 1)])
                           for gg in range(4)])                    # [4, 128, 2N]
            sel = (graph_idx == g).astype(np.float32)[:, None] * mask_f  # [B, N]
            r = ptile(sel.T.astype(bf16))                          # [128, NT*B]
        else:
            xt = np.zeros((2, 128, FT * N // 2), bf16)
            at = np.zeros((4, 128, 2 * N), fp8)
            r = np.zeros((128, NT * B), bf16)
        m = {"xt": xt, "at": at, "w_in": w_in_b, "gw": gw_b, "r": r}
        if use_bias:
            m["bias"] = bias_bcast
        in_maps.append(m)
    return in_maps


def kernel(batch_xs, batch_as, w_in, b_in, gcn_w, gcn_b, graph_idx, cp_mask):
    from concourse import bass_utils

    batch_xs = np.asarray(batch_xs, np.float32)
    batch_as = np.asarray(batch_as, np.float32)
    w_in = np.asarray(w_in, np.float32)
    b_in = np.asarray(b_in, np.float32)
    gcn_w = np.asarray(gcn_w, np.float32)
    gcn_b = np.asarray(gcn_b, np.float32)
    graph_idx = np.asarray(graph_idx).astype(np.int64)
    cp_mask = np.asarray(cp_mask).astype(bool)

    use_bias = bool(np.any(b_in) or np.any(gcn_b))
    nc = _get_nc(use_bias)
    in_maps = _prepare_in_maps(batch_xs, batch_as, w_in, b_in, gcn_w, gcn_b,
                               graph_idx, cp_mask, use_bias)

    res = bass_utils.run_bass_kernel_spmd(nc, in_maps,
                                          core_ids=list(range(N_CORES)))

    partial = np.zeros((B, H), np.float64)
    for c in range(G):
        partial += res.results[c]["out"].astype(np.float64)
    denom = np.maximum(cp_mask.sum(axis=1, keepdims=True).astype(np.float64), 1.0)
    return (partial / denom).astype(np.float32)
